# revision 22
# baseline (speedup 1.0000x reference)
"""Causal self-attention (B=2, L=2048, H=16, D=64) head-sharded over 8 TRN2 cores.

Per core c (local heads {2c, 2c+1}):
  - Host passes x^T (HID, B*L) bf16; all matmuls contract over partitions.
  - QKV^T projection produces d-major tensors with host-permuted Wqkv columns:
      plane A = first-half rope dims  [q1h0|q1h1|k1h0|k1h1] x tok
      plane B = second-half rope dims [q2h0|q2h1|k2h0|k2h1] x tok
      plane C = V^T                   [v_h0(64)|v_h1(64)]   x tok
  - RMSNorm stats via one-hot matmuls; rope as 128-partition DVE bf16 ops
    (2x DVE mode); rope output lands in one [128,2,512] tile so a single
    SBUF->SBUF DMA per half regathers Q^T/K^T (d-interleaved row order,
    consistent between Q and K so scores are unchanged).
  - Engine split in phase 1: PE matmuls (stats deferred one chunk to avoid
    stalls), Pool does PSUM->SBUF copies, ACT does sqrt only, DVE does
    squares + rope in bf16.
  - Scores transposed (k-major): S^T[k,q] = K^T.T @ Q^T, P^T = exp feeds P@V.
    Softmax uses a constant bias; row-sum via appended ones-column on V
    (M=65). Causal-band blocks are trimmed: exp/mask/st/pv only touch the
    valid q-range, and the mask is a single [128,2,128] triangle multiply.
  - Division (1/rowsum) + Wo are pipelined into the attention stream with a
    one-chunk delay so PE never waits on the recip chain. Each core emits a
    full-size bf16 partial out^T (Wo row-shard); host sums.
"""

import numpy as np
import ml_dtypes

import concourse.bacc as bacc
import concourse.bass as bass
import concourse.mybir as mybir
import concourse.tile as tile
from concourse import bass_utils

F32 = mybir.dt.float32
F32R = mybir.dt.float32r
BF16 = mybir.dt.bfloat16

CFG = dict(B=2, L=2048, H=16, D=64, EPS=1e-6)
N_CORES = 8

TOKCH = 512   # token chunk for QKV projection
QCH = 512     # attention q chunk
KBLK = 128    # attention k block


def build_program(cfg, c_bias):
    B, L, H, D = cfg["B"], cfg["L"], cfg["H"], cfg["D"]
    HID = H * D
    BT = B * L
    NTOK = BT // TOKCH          # qkv token chunks
    NHID = HID // 128           # hidden (contraction) chunks
    NQC = L // QCH              # q chunks per batch
    NKB = L // KBLK            # k blocks per batch
    scale = 1.0 / float(np.sqrt(D))
    Exp = mybir.ActivationFunctionType.Exp
    Sqrt = mybir.ActivationFunctionType.Sqrt

    nc = bacc.Bacc("TRN2", target_bir_lowering=False, debug=False,
                   num_devices=N_CORES)

    xT = nc.dram_tensor("xT", [HID, BT], BF16, kind="ExternalInput").ap()
    wqkv = nc.dram_tensor("wqkv", [HID, 384], BF16, kind="ExternalInput").ap()
    wo = nc.dram_tensor("wo", [128, HID], BF16, kind="ExternalInput").ap()
    csT = nc.dram_tensor("csT", [32, L], BF16, kind="ExternalInput").ap()
    snT = nc.dram_tensor("snT", [32, L], BF16, kind="ExternalInput").ap()
    triC = nc.dram_tensor("triC", [128, 128], BF16, kind="ExternalInput").ap()
    identneg = nc.dram_tensor("identneg", [128, 128], BF16, kind="ExternalInput").ap()
    sq_ind = nc.dram_tensor("sq_ind", [128, 4], BF16, kind="ExternalInput").ap()
    wA_ind = nc.dram_tensor("wA_ind", [4, 128], F32R, kind="ExternalInput").ap()
    wB_ind = nc.dram_tensor("wB_ind", [4, 128], F32R, kind="ExternalInput").ap()
    onesc = nc.dram_tensor("onesc", [128, 64], BF16, kind="ExternalInput").ap()
    identd = nc.dram_tensor("identd", [128, 128], BF16, kind="ExternalInput").ap()
    outT = nc.dram_tensor("outT", [HID, BT], BF16, kind="ExternalOutput").ap()

    with tile.TileContext(nc) as tc:
        with tc.tile_pool(name="const", bufs=1) as const, \
             tc.tile_pool(name="big", bufs=1) as big:
            w_sb = const.tile([128, NHID, 384], BF16)
            wo_sb = const.tile([128, HID], BF16)
            cs_sb = const.tile([128, BT], BF16)
            sn_sb = const.tile([128, BT], BF16)
            triC_sb = const.tile([128, 128], BF16)
            identneg_sb = const.tile([128, 128], BF16)
            sqind_sb = const.tile([128, 4], BF16)
            nc.sync.dma_start(out=sqind_sb, in_=sq_ind)
            wAind_sb = const.tile([4, 128], F32R)
            nc.sync.dma_start(out=wAind_sb, in_=wA_ind)
            wBind_sb = const.tile([4, 128], F32R)
            nc.sync.dma_start(out=wBind_sb, in_=wB_ind)
            ident_sb = const.tile([128, 128], BF16)
            ones_sb = const.tile([128, 64], BF16)
            eps_sb = const.tile([128, 1], F32)
            nc.vector.memset(eps_sb, float(cfg["EPS"]))
            cb_sb = const.tile([128, 1], F32)
            nc.vector.memset(cb_sb, -float(c_bias))

            QT = big.tile([128, BT], BF16)
            KT = big.tile([128, BT], BF16)
            Vall = big.tile([128, B * NKB, 130], BF16)
            nc.vector.memset(Vall[:, :, 64:65], 1.0)
            nc.vector.memset(Vall[:, :, 129:130], 1.0)
            attn_div = [big.tile([128, L], BF16, name=f"attn_div{b}")
                        for b in range(B)]
            vtbig = big.tile([128, B, L], BF16, name="vtbig")
            obs = [big.tile([128, B, L], BF16, name=f"ob{o}")
                   for o in range(NHID)]

            # ---------- Phase 1: QKV^T proj + rmsnorm + rope + regather + V
            with tc.tile_pool(name="p1", bufs=2) as p1, \
                 tc.tile_pool(name="qkvps", bufs=2, space="PSUM") as qkvps, \
                 tc.tile_pool(name="scps", bufs=2, space="PSUM") as scps:
                chunks = {}

                def stats_emit(t):
                    ts = slice(t * TOKCH, (t + 1) * TOKCH)
                    ch = chunks.pop(t)
                    var_ps = scps.tile([4, TOKCH], F32, tag="sc", name="var_ps")
                    nc.tensor.matmul(var_ps, sqind_sb, ch["sqA"], start=True, stop=False)
                    nc.tensor.matmul(var_ps, sqind_sb, ch["sqB"], start=False, stop=True)
                    sqv = p1.tile([4, TOKCH], F32, tag="sqv", name="sqv")
                    nc.scalar.activation(sqv, var_ps, Sqrt, bias=eps_sb[0:4])
                    rstd = p1.tile([4, TOKCH], F32R, tag="rstd", name="rstd")
                    with nc.allow_low_precision(reason="f32r rstd"):
                        nc.vector.reciprocal(rstd, sqv)
                    bcA_ps = scps.tile([128, TOKCH], F32, tag="sc", name="bcA_ps")
                    nc.tensor.matmul(bcA_ps, wAind_sb, rstd, start=True, stop=True)
                    bcB_ps = scps.tile([128, TOKCH], F32, tag="sc", name="bcB_ps")
                    nc.tensor.matmul(bcB_ps, wBind_sb, rstd, start=True, stop=True)
                    An = p1.tile([128, TOKCH], BF16, tag="An", name="An")
                    Bn = p1.tile([128, TOKCH], BF16, tag="Bn", name="Bn")
                    with nc.allow_low_precision(reason="bf16 norm"):
                        nc.vector.tensor_mul(An, ch["A"], bcA_ps)
                        nc.vector.tensor_mul(Bn, ch["B"], bcB_ps)
                        # rope on the normed chunk -> one [128,2,TOKCH] tile
                        t12 = p1.tile([128, 2, TOKCH], BF16, tag="t12",
                                      name="t12", bufs=3)
                        tmp = p1.tile([128, TOKCH], BF16, tag="tmp", name="tmp")
                        tmp2 = p1.tile([128, TOKCH], BF16, tag="tmp2", name="tmp2")
                        beng = nc.gpsimd if t < NTOK - 1 else nc.vector
                        beng.tensor_mul(tmp, Bn, sn_sb[:, ts])
                        beng.tensor_mul(tmp2, Bn, cs_sb[:, ts])
                        nc.vector.tensor_mul(t12[:, 0, :], An, cs_sb[:, ts])
                        nc.vector.tensor_sub(t12[:, 0, :], t12[:, 0, :], tmp)
                        nc.vector.tensor_mul(t12[:, 1, :], An, sn_sb[:, ts])
                        nc.vector.tensor_add(t12[:, 1, :], t12[:, 1, :], tmp2)
                    # regather into per-head-contiguous (d-interleaved) Q^T/K^T
                    nc.gpsimd.dma_start(out=QT[:, ts], in_=t12[0:64, :, :])
                    nc.gpsimd.dma_start(out=KT[:, ts], in_=t12[64:128, :, :])
                    if t + 1 == NTOK // B:
                        # batch-0 V transposes; batch 1's happen in phase 2
                        for g in range(0, NKB, 2):
                            vt_ps = scps.tile([128, 2, 128], BF16, tag="sc",
                                              name="vt_ps")
                            for u in range(2):
                                nc.tensor.transpose(
                                    vt_ps[:, u, :],
                                    vtbig[:, 0, 128 * (g + u):128 * (g + u + 1)],
                                    ident_sb)
                            with nc.allow_low_precision(reason="bf16 V"):
                                nc.scalar.copy(Vall[:, g:g + 2, 0:64],
                                               vt_ps[:, :, 0:64])
                                nc.vector.tensor_copy(
                                    Vall[:, g:g + 2, 65:129],
                                    vt_ps[:, :, 64:128])

                xTv = xT.rearrange("(k p) q -> p k q", p=128)
                for t in range(NTOK):
                    ts = slice(t * TOKCH, (t + 1) * TOKCH)
                    xt = p1.tile([128, NHID, TOKCH], BF16, tag="xt",
                                 name="xt", bufs=4)
                    if t == 0:
                        for k in range(NHID):
                            nc.sync.dma_start(out=w_sb[:, k, :],
                                              in_=wqkv[128 * k:128 * (k + 1), :])
                            nc.sync.dma_start(out=xt[:, k, :],
                                              in_=xTv[:, k, ts])
                    else:
                        nc.sync.dma_start(out=xt, in_=xTv[:, :, ts])
                    if t == 0:
                        # deferred bulk loads: queued after the first x tiles
                        for tab, src in ((cs_sb, csT), (sn_sb, snT)):
                            for b in range(B):
                                nc.sync.dma_start(
                                    out=tab[0:32, b * L:(b + 1) * L], in_=src)
                            nc.sync.dma_start(out=tab[32:64, :], in_=tab[0:32, :])
                            nc.sync.dma_start(out=tab[64:128, :], in_=tab[0:64, :])
                        nc.sync.dma_start(out=triC_sb, in_=triC)
                        nc.sync.dma_start(out=identneg_sb, in_=identneg)
                        nc.sync.dma_start(out=ident_sb, in_=identd)
                        nc.sync.dma_start(out=ones_sb, in_=onesc)
                        nc.sync.dma_start(out=wo_sb, in_=wo)
                    qkv_ps = qkvps.tile([128, 3, TOKCH], F32, tag="qkv",
                                        name="qkv_ps")
                    for k in range(NHID):
                        for m in range(3):
                            nc.tensor.matmul(
                                qkv_ps[:, m, :],
                                w_sb[:, k, 128 * m:128 * (m + 1)],
                                xt[:, k, :],
                                start=(k == 0), stop=(k == NHID - 1))
                    # deferred stats for the previous chunk keep PE busy here
                    if t > 0:
                        stats_emit(t - 1)
                    A_sb = p1.tile([128, TOKCH], BF16, tag="A", name="A_sb",
                                   bufs=3)
                    B_sb = p1.tile([128, TOKCH], BF16, tag="B", name="B_sb",
                                   bufs=3)
                    with nc.allow_low_precision(reason="bf16 qkv"):
                        nc.scalar.copy(A_sb, qkv_ps[:, 0, :])
                        nc.scalar.copy(B_sb, qkv_ps[:, 1, :])
                        nc.vector.tensor_copy(
                            vtbig[:, t // (NTOK // B),
                                  (t % (NTOK // B)) * TOKCH:
                                  (t % (NTOK // B) + 1) * TOKCH],
                            qkv_ps[:, 2, :])
                        sqA = p1.tile([128, TOKCH], BF16, tag="sqA", name="sqA")
                        sqB = p1.tile([128, TOKCH], BF16, tag="sqB", name="sqB")
                        nc.vector.tensor_mul(sqA, A_sb, A_sb)
                        nc.vector.tensor_mul(sqB, B_sb, B_sb)
                    chunks[t] = dict(A=A_sb, B=B_sb, sqA=sqA, sqB=sqB)
                stats_emit(NTOK - 1)

            # ---------- Phase 2: attention + division + Wo, pipelined
            with tc.tile_pool(name="p2", bufs=2) as p2, \
                 tc.tile_pool(name="stps", bufs=2, space="PSUM") as stps, \
                 tc.tile_pool(name="pvps", bufs=2, space="PSUM") as pvps:

                def division(b, j):
                    js = slice(j * QCH, (j + 1) * QCH)
                    pv = pv_tiles[(b, j)]
                    bcs = []
                    for h in range(2):
                        rec = p2.tile([65, QCH], BF16, tag=f"rec{h}",
                                      name=f"rec{h}")
                        with nc.allow_low_precision(reason="bf16 rowsum recip"):
                            nc.vector.reciprocal(rec[64:65, :], pv[h][64:65, :])
                        bc = pvps.tile([64, QCH], F32, tag=f"pv{h}", name="bc")
                        nc.tensor.matmul(bc, ones_sb[64:65, :],
                                         rec[64:65, :], start=True, stop=True)
                        bcsb = p2.tile([64, QCH], BF16, tag=f"bcs{h}",
                                       name=f"bcs{h}")
                        with nc.allow_low_precision(reason="bf16 bc"):
                            nc.vector.tensor_copy(bcsb, bc)
                        bcs.append(bcsb)
                    with nc.allow_low_precision(reason="bf16 attn"):
                        nc.vector.tensor_mul(attn_div[b][0:64, js],
                                             bcs[0], pv[0][0:64, :])
                        h1t = p2.tile([64, QCH], BF16, tag="h1t", name="h1t")
                        nc.vector.tensor_mul(h1t, bcs[1], pv[1][0:64, :])
                    nc.gpsimd.dma_start(out=attn_div[b][64:128, js], in_=h1t)

                def wo_emit(b, j):
                    js = slice(j * QCH, (j + 1) * QCH)
                    for o in range(NHID):
                        wops = pvps.tile([128, QCH], F32, tag=f"pv{o % 2}",
                                         name="wops")
                        nc.tensor.matmul(wops, wo_sb[:, 128 * o:128 * (o + 1)],
                                         attn_div[b][:, js],
                                         start=True, stop=True)
                        with nc.allow_low_precision(reason="bf16 out"):
                            if o % 4 == 3:
                                nc.scalar.copy(obs[o][:, b, js], wops)
                            else:
                                nc.vector.tensor_copy(obs[o][:, b, js], wops)
                    if j == NQC - 1:
                        for o in range(NHID):
                            dma = nc.sync if o % 2 == 0 else nc.gpsimd
                            dma.dma_start(
                                out=outT[128 * o:128 * (o + 1),
                                         b * L:(b + 1) * L],
                                in_=obs[o][:, b, :])

                pv_tiles = {}
                prev = None
                for b in range(B):
                    for j in range(NQC):
                        qs = slice(b * L + j * QCH, b * L + (j + 1) * QCH)
                        pv = [pvps.tile([65, QCH], F32, tag=f"pv{h}",
                                        name=f"pv{h}") for h in range(2)]
                        pv_tiles[(b, j)] = pv
                        nkb = (QCH // KBLK) * (j + 1)
                        for i in range(nkb):
                            s_off = KBLK * i - QCH * j
                            q0 = max(s_off, 0)
                            st = stps.tile([128, 2, QCH], F32, tag="st",
                                           name="st")
                            for h in range(2):
                                nc.tensor.matmul(
                                    st[:, h, q0:],
                                    KT[64 * h:64 * (h + 1),
                                       b * L + KBLK * i: b * L + KBLK * (i + 1)],
                                    QT[64 * h:64 * (h + 1),
                                       qs.start + q0:qs.stop],
                                    start=True, stop=s_off < 0)
                            if s_off >= 0:
                                # causal mask folded into the scores: add
                                # -BIG to the invalid triangle via a second
                                # accumulating matmul (53ns on PE)
                                for h in range(2):
                                    nc.tensor.matmul(
                                        st[:, h, q0:q0 + KBLK],
                                        identneg_sb, triC_sb,
                                        start=False, stop=True)
                            pexp = p2.tile([128, 2, QCH], BF16, tag="pexp",
                                           name="pexp", bufs=4)
                            nc.scalar.activation(
                                pexp[:, :, q0:], st[:, :, q0:],
                                Exp, bias=cb_sb, scale=scale)
                            for h in range(2):
                                nc.tensor.matmul(
                                    pv[h][:, q0:],
                                    Vall[:, b * NKB + i, 65 * h:65 * (h + 1)],
                                    pexp[:, h, q0:],
                                    start=(i == 0), stop=(i == nkb - 1))
                            if (b, j) == (0, 1):
                                # batch-1 V transposes, hidden in this i-loop
                                vt_ps = stps.tile([128, 2, 128], BF16,
                                                  tag="st", name="vt_ps")
                                for u in range(2):
                                    nc.tensor.transpose(
                                        vt_ps[:, u, :],
                                        vtbig[:, 1, 128 * (2 * i + u):
                                              128 * (2 * i + u + 1)],
                                        ident_sb)
                                blk = NKB + 2 * i
                                with nc.allow_low_precision(reason="bf16 V"):
                                    nc.scalar.copy(Vall[:, blk:blk + 2, 0:64],
                                                   vt_ps[:, :, 0:64])
                                    nc.vector.tensor_copy(
                                        Vall[:, blk:blk + 2, 65:129],
                                        vt_ps[:, :, 64:128])
                        # Wo of the previous chunk first so PE never waits on
                        # this chunk's recip chain
                        if prev is not None:
                            wo_emit(*prev)
                        division(b, j)
                        prev = (b, j)
                wo_emit(*prev)
    nc.compile()
    return nc


def prep_inputs(inputs, cfg):
    B, L, H, D = cfg["B"], cfg["L"], cfg["H"], cfg["D"]
    HID = H * D
    BT = B * L
    x = np.asarray(inputs["x"], np.float32)
    Wqkv = np.asarray(inputs["Wqkv"], np.float32)
    Wo = np.asarray(inputs["Wo"], np.float32)
    qw = np.asarray(inputs["q_norm_w"], np.float32)
    kw = np.asarray(inputs["k_norm_w"], np.float32)
    cos = np.asarray(inputs["cos"], np.float32)[:L]
    sin = np.asarray(inputs["sin"], np.float32)[:L]

    xT = np.ascontiguousarray(x.reshape(BT, HID).T).astype(ml_dtypes.bfloat16)
    csT = np.ascontiguousarray(cos.T).astype(ml_dtypes.bfloat16)
    snT = np.ascontiguousarray(sin.T).astype(ml_dtypes.bfloat16)
    ki = np.arange(128)[:, None]
    mm = np.arange(128)[None, :]
    triC = (mm < ki).astype(ml_dtypes.bfloat16)           # invalid triangle
    identneg = (-1e4 * np.eye(128)).astype(ml_dtypes.bfloat16)
    sq_ind = np.zeros((128, 4), np.float32)
    sq_ind[np.arange(128), np.arange(128) // 32] = 1.0 / D
    sq_ind = sq_ind.astype(ml_dtypes.bfloat16)
    wA = np.zeros((4, 128), np.float32)
    wB = np.zeros((4, 128), np.float32)
    d2 = D // 2
    for m, w in enumerate([qw, qw, kw, kw]):
        cols = np.arange(32) + 32 * m
        wA[m, cols] = w[:d2]
        wB[m, cols] = w[d2:]
    c_bias = float(np.sqrt(D) * max(np.abs(qw).max() * np.abs(kw).max(), 1e-6))

    hpc = H // N_CORES
    in_maps = []
    for c in range(N_CORES):
        h0 = hpc * c
        h1 = h0 + 1
        d32 = np.arange(d2)
        Acols = np.r_[h0 * D + d32, h1 * D + d32,
                      HID + h0 * D + d32, HID + h1 * D + d32]
        Bcols = Acols + d2
        Ccols = np.r_[2 * HID + h0 * D + np.arange(D),
                      2 * HID + h1 * D + np.arange(D)]
        w_c = np.ascontiguousarray(Wqkv[:, np.r_[Acols, Bcols, Ccols]]).astype(ml_dtypes.bfloat16)
        wo_c = np.ascontiguousarray(Wo[128 * c:128 * (c + 1), :]).astype(ml_dtypes.bfloat16)
        in_maps.append(dict(xT=xT, wqkv=w_c, wo=wo_c, csT=csT, snT=snT,
                            triC=triC, identneg=identneg, sq_ind=sq_ind,
                            wA_ind=wA, wB_ind=wB,
                            onesc=np.ones((128, 64), ml_dtypes.bfloat16),
                            identd=np.eye(128, dtype=ml_dtypes.bfloat16)))
    return in_maps, c_bias


def gather_output(results, cfg):
    B, L, H, D = cfg["B"], cfg["L"], cfg["H"], cfg["D"]
    HID = H * D
    acc = np.zeros((HID, B * L), np.float32)
    for r in results:
        acc += r["outT"].astype(np.float32)
    return np.ascontiguousarray(acc.T).reshape(B, L, HID).astype(np.float32)


def kernel(**inputs):
    in_maps, c_bias = prep_inputs(inputs, CFG)
    nc = build_program(CFG, c_bias)
    res = bass_utils.run_bass_kernel_spmd(nc, in_maps, core_ids=list(range(N_CORES)))
    return gather_output(res.results, CFG)
